# revision 7
# baseline (speedup 1.0000x reference)
"""Linear attention (elu+1 feature map) Trainium2 Bass kernel.

Sharding: 8 cores = (batch 4) x (sequence halves 2). Each core computes the
full pipeline (QKV projections, chunked causal linear attention, output
projection) for its (batch, seq-half) slice. The only cross-core dependency
is the running KV' prefix state at the half boundary, exchanged with a
pair-wise AllReduce (left core contributes its total state, right core
consumes it; masking by per-core side flags keeps the program SPMD-uniform).

Data path: bf16 matmul operands with fp32 PSUM accumulation everywhere.

Chunked linear attention per (head, chunk of 128):
    A^T[j,i] = k_j . q_i            (PE, lhsT=K^T rhs=Q^T)
    A_m^T    = A^T * triu_mask      (DVE; doubles as the PSUM->SBUF move)
    out_pre  = A_m^T.T @ V' + Q @ S (PE, accumulated in one PSUM region)
    den      = out_pre[:, 64] + eps; attn = out_pre[:, :64] / den
    S       += K^T @ V'             (PE, fp32 PSUM-resident accumulation)
where V' = [V | 1] (65 cols) so numerator and denominator share matmuls.
"""

import os
import sys

import numpy as np

for _p in ("/opt/trn_rl_repo", "/root/.axon_site/_ro/trn_rl_repo"):
    if os.path.isdir(_p) and _p not in sys.path:
        sys.path.append(_p)

import ml_dtypes  # noqa: E402

import concourse.bass as bass  # noqa: E402
import concourse.tile as tile  # noqa: E402
from concourse import bacc, mybir  # noqa: E402
from concourse.bass_utils import run_bass_kernel_spmd  # noqa: E402

F32 = mybir.dt.float32
F32R = mybir.dt.float32r
BF16 = mybir.dt.bfloat16
AF = mybir.ActivationFunctionType
ALU = mybir.AluOpType

B, S, HID = 4, 4096, 1024
H, D = 16, 64
E = D + 1          # V' columns (extra ones column -> denominator)
C = 128            # chunk (sequence tile)
SH = S // 2        # rows per core
NCH = SH // C      # 16 chunks per core
BLK = 512          # projection block (4 chunks)
NBLK = SH // BLK
KP = HID // 128    # contraction k-tiles
PT = (H * D) // 128  # head-dim partition tiles
EPS = 1e-6

# head -> (psum group, offset) packing: 7 + 7 + 2 heads per 2KB PSUM bank
GRP = [(h // 7, h % 7) for h in range(H)]
NG = [7, 7, 2]
GSIZE = [n * E for n in NG]
GOFF = [0, 7, 14]
REPLICA_GROUPS = [[0, 1], [2, 3], [4, 5], [6, 7]]


def build_program():
    nc = bacc.Bacc("TRN2", target_bir_lowering=False, debug=False, num_devices=8)

    # ---- DRAM I/O -------------------------------------------------------
    xs = nc.dram_tensor("xs", [SH, HID], BF16, kind="ExternalInput").ap()
    wq = nc.dram_tensor("wq", [HID, H * D], BF16, kind="ExternalInput").ap()
    wk = nc.dram_tensor("wk", [HID, H * D], BF16, kind="ExternalInput").ap()
    wv = nc.dram_tensor("wv", [HID, H * D], BF16, kind="ExternalInput").ap()
    wo = nc.dram_tensor("wo", [H * D, HID], BF16, kind="ExternalInput").ap()
    bq = nc.dram_tensor("bq", [128, PT], F32, kind="ExternalInput").ap()
    bk = nc.dram_tensor("bk", [128, PT], F32, kind="ExternalInput").ap()
    bv = nc.dram_tensor("bv", [1, H * D], BF16, kind="ExternalInput").ap()
    bo = nc.dram_tensor("bo", [1, HID], BF16, kind="ExternalInput").ap()
    maskd = nc.dram_tensor("maskf", [C, C], F32, kind="ExternalInput").ap()
    identd = nc.dram_tensor("ident", [D, D], BF16, kind="ExternalInput").ap()
    flagsd = nc.dram_tensor("flags", [1, 2], BF16, kind="ExternalInput").ap()
    out_f = nc.dram_tensor("out_f", [SH, HID], F32, kind="ExternalOutput").ap()

    with tile.TileContext(nc) as tc, tc.tile_pool(name="const", bufs=1) as cpool, \
            tc.tile_pool(name="resid", bufs=1) as rpool, \
            tc.tile_pool(name="dram", bufs=1, space="DRAM") as dpool, \
            tc.tile_pool(name="stpsum", bufs=1, space="PSUM") as stpool:

        # ---- persistent small constants --------------------------------
        sb_bq = cpool.tile([128, PT], F32, tag="bq")
        sb_bk = cpool.tile([128, PT], F32, tag="bk")
        nc.sync.dma_start(sb_bq[:], bq[:])
        nc.sync.dma_start(sb_bk[:], bk[:])
        sb_mask = cpool.tile([C, C], F32, tag="mask")
        nc.sync.dma_start(sb_mask[:], maskd[:])
        bv_bc = cpool.tile([128, H * D], F32, tag="bv_bc")
        bo_bc = cpool.tile([128, HID], F32, tag="bo_bc")
        fl_bc = cpool.tile([D, 2], F32, tag="fl_bc")

        # broadcast bias rows / flags across partitions with K=1 f32r matmuls
        with tc.tile_pool(name="init", bufs=1) as ipool, \
                tc.tile_pool(name="bcps", bufs=2, space="PSUM") as bcps:
            sb_bv1 = ipool.tile([1, H * D], BF16, tag="bv1")
            sb_bo1 = ipool.tile([1, HID], BF16, tag="bo1")
            sb_fl1 = ipool.tile([1, 2], BF16, tag="fl1")
            ones1 = ipool.tile([1, 128], BF16, tag="ones1")
            nc.sync.dma_start(sb_bv1[:], bv[:])
            nc.sync.dma_start(sb_bo1[:], bo[:])
            nc.sync.dma_start(sb_fl1[:], flagsd[:])
            nc.vector.memset(ones1[:], 1.0)
            for n2 in range(2):
                t = bcps.tile([128, 512], F32, tag="bc")
                nc.tensor.matmul(t[:], ones1[:, :128],
                                 sb_bv1[:, n2 * 512:(n2 + 1) * 512],
                                 start=True, stop=True)
                nc.scalar.copy(bv_bc[:, n2 * 512:(n2 + 1) * 512], t[:])
                t2 = bcps.tile([128, 512], F32, tag="bc")
                nc.tensor.matmul(t2[:], ones1[:, :128],
                                 sb_bo1[:, n2 * 512:(n2 + 1) * 512],
                                 start=True, stop=True)
                nc.scalar.copy(bo_bc[:, n2 * 512:(n2 + 1) * 512], t2[:])
            tf = bcps.tile([D, 2], F32, tag="fl")
            nc.tensor.matmul(tf[:], ones1[:, :D],
                             sb_fl1[:], start=True, stop=True)
            nc.scalar.copy(fl_bc[:], tf[:])

        # ---- resident activations --------------------------------------
        qT = [rpool.tile([128, SH], BF16, tag=f"qT{p}", name=f"qT{p}") for p in range(PT)]
        kT = [rpool.tile([128, SH], BF16, tag=f"kT{p}", name=f"kT{p}") for p in range(PT)]
        vP = [rpool.tile([128, H, E], BF16, tag=f"vP{r}", name=f"vP{r}") for r in range(NCH)]

        # prefix-state PSUM tiles (fp32, live across the whole kernel)
        st = [stpool.tile([D, GSIZE[g]], F32, tag=f"st{g}", name=f"st{g}") for g in range(3)]

        # ---- phase 1: projections + phi + local KV state ----------------
        with tc.tile_pool(name="wqk", bufs=1) as wpool, \
                tc.tile_pool(name="p1ps", bufs=3, space="PSUM") as pps, \
                tc.tile_pool(name="p1tmp", bufs=2) as tpool, \
                tc.tile_pool(name="p1kn", bufs=2) as knpool:
            wq_t = [wpool.tile([128, H * D], BF16, tag=f"wq{p}", name=f"wq{p}") for p in range(KP)]
            wk_t = [wpool.tile([128, H * D], BF16, tag=f"wk{p}", name=f"wk{p}") for p in range(KP)]
            wv_t = [wpool.tile([128, H * D], BF16, tag=f"wv{p}", name=f"wv{p}") for p in range(KP)]
            for p in range(KP):
                nc.sync.dma_start(wq_t[p][:], wq[p * 128:(p + 1) * 128, :])
                nc.sync.dma_start(wk_t[p][:], wk[p * 128:(p + 1) * 128, :])
                nc.sync.dma_start(wv_t[p][:], wv[p * 128:(p + 1) * 128, :])

            for blk in range(NBLK):
                bs = blk * BLK
                xt = tpool.tile([128, KP, BLK], BF16, tag="xt")
                for kp in range(KP):
                    nc.sync.dma_start_transpose(
                        xt[:, kp, :], xs[bs:bs + BLK, kp * 128:(kp + 1) * 128])

                # K^T first (so the last block's KV state is ready earliest),
                # then Q^T, then V.
                for (wt, bias, dst) in ((wk_t, sb_bk, kT), (wq_t, sb_bq, qT)):
                    for p in range(PT):
                        ps = pps.tile([128, BLK], F32, tag="proj")
                        for kp in range(KP):
                            nc.tensor.matmul(
                                ps[:], wt[kp][:, p * 128:(p + 1) * 128],
                                xt[:, kp, :], start=(kp == 0), stop=(kp == KP - 1))
                        # phi(z) = max(z+b, 0) + exp(min(z+b, 0))
                        ct = tpool.tile([128, BLK], F32, tag="phic")
                        nc.scalar.activation(ct[:], ps[:], AF.Identity,
                                             bias=bias[:, p:p + 1], scale=1.0)
                        at = tpool.tile([128, BLK], F32, tag="phiae")
                        nc.vector.tensor_scalar_min(at[:], ct[:], 0.0)
                        et = tpool.tile([128, BLK], F32, tag="phiae")
                        nc.scalar.activation(et[:], at[:], AF.Exp)
                        nc.vector.scalar_tensor_tensor(
                            dst[p][:, bs:bs + BLK], ct[:], 0.0, et[:],
                            op0=ALU.max, op1=ALU.add)

                for r in range(BLK // C):
                    rg = blk * (BLK // C) + r
                    for n2 in range(2):
                        ps = pps.tile([128, 512], F32, tag="proj")
                        for kp in range(KP):
                            nc.tensor.matmul(
                                ps[:], xt[:, kp, r * C:(r + 1) * C],
                                wv_t[kp][:, n2 * 512:(n2 + 1) * 512],
                                start=(kp == 0), stop=(kp == KP - 1))
                        nc.vector.scalar_tensor_tensor(
                            vP[rg][:, n2 * 8:(n2 + 1) * 8, 0:D],
                            ps.rearrange("p (h d) -> p h d", d=D), 0.0,
                            bv_bc[:, n2 * 512:(n2 + 1) * 512]
                                .rearrange("p (h d) -> p h d", d=D),
                            op0=ALU.add, op1=ALU.add)
                    nc.vector.memset(vP[rg][:, :, D:E], 1.0)

                # local KV' state for this block's chunks (phase 2, inlined)
                for r in range(BLK // C):
                    ch = blk * (BLK // C) + r
                    kn = knpool.tile([128, PT, C], BF16, tag="kn")
                    for p in range(PT):
                        nc.sync.dma_start_transpose(
                            kn[:, p, :], kT[p][:, ch * C:(ch + 1) * C])
                    knf = kn.rearrange("p a b -> p (a b)")
                    for h in range(H):
                        g, o = GRP[h]
                        nc.tensor.matmul(
                            st[g][:, o * E:(o + 1) * E],
                            knf[:, h * D:(h + 1) * D], vP[ch][:, h, :],
                            start=(ch == 0 and o == 0),
                            stop=(ch == NCH - 1 and o == NG[g] - 1),
                            skip_group_check=True)

        # ---- state exchange (pair AllReduce, masked by side flags) ------
        with tc.tile_pool(name="xchg", bufs=1) as xpool:
            sb_id = xpool.tile([D, D], BF16, tag="ident")
            nc.sync.dma_start(sb_id[:], identd[:])
            s_loc = xpool.tile([D, H * E], F32, tag="s_loc")
            for g in range(3):
                nc.scalar.copy(s_loc[:, GOFF[g] * E:GOFF[g] * E + GSIZE[g]],
                               st[g][:])
            s_msk = xpool.tile([D, H * E], F32, tag="s_msk")
            nc.vector.tensor_scalar_mul(s_msk[:], s_loc[:], fl_bc[:, 0:1])
            cc_in = dpool.tile([D, H * E], F32, tag="cc_in")
            cc_out = dpool.tile([D, H * E], F32, tag="cc_out")
            nc.sync.dma_start(cc_in[:], s_msk[:])
            nc.gpsimd.collective_compute(
                "AllReduce", ALU.add, replica_groups=REPLICA_GROUPS,
                ins=[cc_in.opt()], outs=[cc_out.opt()])
            s_rem = xpool.tile([D, H * E], F32, tag="s_rem")
            nc.sync.dma_start(s_rem[:], cc_out[:])
            s_remm = xpool.tile([D, H * E], F32, tag="s_remm")
            nc.vector.tensor_scalar_mul(s_remm[:], s_rem[:], fl_bc[:, 1:2])
            s_remb = xpool.tile([D, H * E], BF16, tag="s_remb")
            nc.scalar.copy(s_remb[:], s_remm[:])

            # re-init PSUM state to the remote prefix (identity matmul)
            for g in range(3):
                nc.tensor.matmul(st[g][:], sb_id[:],
                                 s_remb[:, GOFF[g] * E:GOFF[g] * E + GSIZE[g]],
                                 start=True, stop=True, skip_group_check=True)

        # ---- phase 4: attention + output projection ---------------------
        with tc.tile_pool(name="p4wo", bufs=1) as wopool, \
                tc.tile_pool(name="p4ps", bufs=2, space="PSUM") as aps_pool, \
                tc.tile_pool(name="p4op", bufs=1, space="PSUM") as ops_pool, \
                tc.tile_pool(name="p4sb", bufs=2) as spool, \
                tc.tile_pool(name="p4kn", bufs=2) as knpool2:
            wo_t = [wopool.tile([128, HID], BF16, tag=f"wo{p}", name=f"wo{p}") for p in range(PT)]
            for p in range(PT):
                nc.sync.dma_start(wo_t[p][:], wo[p * 128:(p + 1) * 128, :])

            for ch in range(NCH):
                cs = ch * C
                kn = knpool2.tile([128, PT, C], BF16, tag="kn4")
                for p in range(PT):
                    nc.sync.dma_start_transpose(
                        kn[:, p, :], kT[p][:, cs:cs + C])
                knf = kn.rearrange("p a b -> p (a b)")

                # snapshot of the prefix state (bf16, matmul operand),
                # duplicated into both partition halves so each head's inter
                # matmul finds it at the same base partition as its Q^T slice
                sn = spool.tile([128, H * E], BF16, tag="snap")
                for g in range(3):
                    nc.scalar.copy(sn[0:D, GOFF[g] * E:GOFF[g] * E + GSIZE[g]],
                                   st[g][:])
                    nc.scalar.copy(sn[D:2 * D, GOFF[g] * E:GOFF[g] * E + GSIZE[g]],
                                   st[g][:])

                op = [ops_pool.tile([128, GSIZE[g]], F32, tag=f"op{g}", name=f"op{g}")
                      for g in range(3)]
                for h in range(H):
                    g, o = GRP[h]
                    p, row = h // 2, (h % 2) * D
                    aps = aps_pool.tile([C, C], F32, tag="apsum")
                    nc.tensor.matmul(
                        aps[:], kT[p][row:row + D, cs:cs + C],
                        qT[p][row:row + D, cs:cs + C], start=True, stop=True)
                    amt = spool.tile([C, C], BF16, tag="amt")
                    nc.vector.tensor_mul(amt[:], aps[:], sb_mask[:])
                    nc.tensor.matmul(op[g][:, o * E:(o + 1) * E], amt[:],
                                     vP[ch][:, h, :], start=True, stop=False,
                                     skip_group_check=True)
                    nc.tensor.matmul(op[g][:, o * E:(o + 1) * E],
                                     qT[p][row:row + D, cs:cs + C],
                                     sn[row:row + D, h * E:(h + 1) * E],
                                     start=False, stop=True,
                                     skip_group_check=True)
                    # KV' state accumulation (after the inter matmul read sn)
                    nc.tensor.matmul(st[g][:, o * E:(o + 1) * E],
                                     knf[:, h * D:(h + 1) * D], vP[ch][:, h, :],
                                     start=False,
                                     stop=(ch == NCH - 1 and o == NG[g] - 1),
                                     skip_group_check=True)

                den = spool.tile([C, H], F32, tag="den")
                for g in range(3):
                    nc.vector.tensor_scalar_add(
                        den[:, GOFF[g]:GOFF[g] + NG[g]],
                        op[g].rearrange("p (n e) -> p n e", e=E)[:, :, D], EPS)
                rec = spool.tile([C, H], F32, tag="rec")
                nc.vector.reciprocal(rec[:], den[:])

                oat = spool.tile([C, H * D], BF16, tag="oat")
                for h in range(H):
                    g, o = GRP[h]
                    src = op[g][:, o * E:o * E + D]
                    if h % 2 == 0:
                        nc.scalar.mul(oat[:, h * D:(h + 1) * D], src,
                                      rec[:, h:h + 1])
                    else:
                        nc.vector.tensor_scalar_mul(
                            oat[:, h * D:(h + 1) * D], src, rec[:, h:h + 1])

                ot = spool.tile([128, PT, C], BF16, tag="ot")
                for p in range(PT):
                    nc.sync.dma_start_transpose(
                        ot[:, p, :], oat[:, p * 128:(p + 1) * 128])
                for n2 in range(2):
                    pso = aps_pool.tile([C, 512], F32, tag="apsum")
                    for p in range(PT):
                        nc.tensor.matmul(pso[:], ot[:, p, :],
                                         wo_t[p][:, n2 * 512:(n2 + 1) * 512],
                                         start=(p == 0), stop=(p == PT - 1))
                    fin = spool.tile([C, 512], F32, tag="fin")
                    nc.vector.tensor_add(fin[:], pso[:],
                                         bo_bc[:, n2 * 512:(n2 + 1) * 512])
                    nc.sync.dma_start(
                        out_f[cs:cs + C, n2 * 512:(n2 + 1) * 512], fin[:])
    nc.compile()
    return nc


_NC_CACHE = None


def _get_nc():
    global _NC_CACHE
    if _NC_CACHE is None:
        _NC_CACHE = build_program()
    return _NC_CACHE


def _bf(a):
    return np.ascontiguousarray(np.asarray(a, np.float32).astype(ml_dtypes.bfloat16))


def _f32(a):
    return np.ascontiguousarray(np.asarray(a, np.float32))


def make_in_maps(x, Wq, bq, Wk, bk, Wv, bv, Wo, bo):
    x = np.asarray(x, np.float32)
    wq_b, wk_b, wv_b, wo_b = _bf(Wq), _bf(Wk), _bf(Wv), _bf(Wo)
    bq_t = _f32(bq).reshape(PT, 128).T.copy()
    bk_t = _f32(bk).reshape(PT, 128).T.copy()
    bv_r = _bf(bv).reshape(1, H * D)
    bo_r = _bf(bo).reshape(1, HID)
    maskf = np.triu(np.ones((C, C), np.float32))
    ident = np.eye(D, dtype=np.float32).astype(ml_dtypes.bfloat16)

    in_maps = []
    for core in range(8):
        b, half = core // 2, core % 2
        xs_i = _bf(x[b, half * SH:(half + 1) * SH, :])
        flags = np.array([[1.0 - half, float(half)]], np.float32).astype(ml_dtypes.bfloat16)
        in_maps.append({
            "xs": xs_i, "wq": wq_b, "wk": wk_b, "wv": wv_b, "wo": wo_b,
            "bq": bq_t, "bk": bk_t, "bv": bv_r, "bo": bo_r,
            "maskf": maskf, "ident": ident, "flags": flags,
        })
    return in_maps


def assemble_out(results):
    out = np.zeros((B, S, HID), np.float32)
    for core in range(8):
        b, half = core // 2, core % 2
        out[b, half * SH:(half + 1) * SH, :] = results[core]["out_f"]
    return out


def kernel(x, Wq, bq, Wk, bk, Wv, bv, Wo, bo):
    in_maps = make_in_maps(x, Wq, bq, Wk, bk, Wv, bv, Wo, bo)
    res = run_bass_kernel_spmd(_get_nc(), in_maps, core_ids=list(range(8)))
    return assemble_out(res.results)


if __name__ == "__main__":
    rng = np.random.default_rng(0)
    demo = {
        "x": rng.standard_normal((B, S, HID)).astype(np.float32),
        "Wq": (rng.standard_normal((HID, H * D)) / 32).astype(np.float32),
        "bq": np.zeros(H * D, np.float32),
        "Wk": (rng.standard_normal((HID, H * D)) / 32).astype(np.float32),
        "bk": np.zeros(H * D, np.float32),
        "Wv": (rng.standard_normal((HID, H * D)) / 32).astype(np.float32),
        "bv": np.zeros(H * D, np.float32),
        "Wo": (rng.standard_normal((H * D, HID)) / 32).astype(np.float32),
        "bo": np.zeros(HID, np.float32),
    }
    o = kernel(**demo)
    print("kernel ran, out shape", o.shape, "finite:", bool(np.isfinite(o).all()))
